# revision 11
# baseline (speedup 1.0000x reference)
"""Trainium2 Bass kernel for AdaptiveSemiseparableLayer.

Reference computation (B=4, L=2048, D=R=2048, DH=512):
    t_out = depthwise_conv1d(x, conv_w, k=3) + conv_b
    u = x @ Wu.T + bu ; v = x @ Wv.T + bv
    gates = sigmoid(relu(x @ Wg1.T + bg1) @ Wg2.T + bg2)
    cs = cumsum(u * gates, axis=seq)
    out = t_out + (cs * (v * gates)) @ Wu.T        # NB: Wu.T indexed [r, d]

Sharding: sequence-parallel. 8192 tokens are split into 8 contiguous
1024-token blocks (each block lies inside one batch row, B=4 * 2 halves),
one block per NeuronCore. All matmuls are purely local; the only
cross-core dependency is the cumsum carry: core c (odd) needs the total
sum of u*gates over core c-1's block. That is an (R,)=8KB AllGather,
folded back into the cumsum PSUM accumulation as one K=8 matmul against
a per-core host-provided mask.

Layouts (per core, T=1024 local tokens):
    xT   [D, T+2]  bf16   transposed shard with conv halo columns
    u, gates        natural  [token-part, r-free]   (ug feeds cumsum as lhsT)
    h, v, gatesT    transposed [feat-part, t-free]
    csT = matmul(lhsT=ug_natural, rhs=tri)  ->  [r-part, t-free] directly
    out = (WuT as lhsT) @ goT -> [d-part, t-free], conv fused in epilogue
"""

import numpy as np
import ml_dtypes
from contextlib import ExitStack

import concourse.bass as bass
import concourse.mybir as mybir
import concourse.tile as tile
from concourse.bass_utils import run_bass_kernel_spmd

P = 128
B, L, D = 4, 2048, 2048
R, DH = 2048, 512
NCORES = 8
T = (B * L) // NCORES          # 1024 tokens per core
TH = T + 2                     # with conv halo
ND, NR, NDH, NT = D // P, R // P, DH // P, T // P
TC = 512                       # matmul free-dim chunk (one PSUM bank of f32)
NTC = T // TC                  # 2
NRC = R // TC                  # 4
TRI_OFF = 384                  # master tri: tri[s, q] = (s <= q - TRI_OFF)
TRI_W = TRI_OFF + TC + TC      # 1408; cs rhs slices tri[:, off+384 : off+896]
ONES_COL = TRI_OFF + TC - 1    # column 895: all ones (s <= 511)
BF = mybir.dt.bfloat16
F32 = mybir.dt.float32
AF = mybir.ActivationFunctionType
ALU = mybir.AluOpType

import os
_PHASES = int(os.environ.get("K_PHASES", "4"))


def _emit(nc, io):
    ctx = ExitStack()
    tc = io["tc"]
    pool = lambda name, bufs, **kw: ctx.enter_context(
        tc.tile_pool(name=name, bufs=bufs, **kw)
    )
    const = pool("const", 1)
    xpool = pool("xpool", ND)
    hpool = pool("hpool", NDH)
    gnp = pool("gnp", NT)
    ugp = pool("ugp", NT * NRC)
    gtp = pool("gtp", 2)
    vgp = pool("vgp", 2)
    gop = pool("gop", NR)
    wg1p = pool("wg1p", 3)
    wg2p = pool("wg2p", 3)
    wg2tp = pool("wg2tp", 3)
    wvtp = pool("wvtp", 3)
    wutp = pool("wutp", 3)
    wu2p = pool("wu2p", 6)
    outp = pool("outp", 2)
    ctp = pool("ctp", 2)
    psum = pool("psum", 8, space="PSUM")
    dram = pool("dram", 1, space="DRAM")

    xT, WuT, WvT, Wg1T, Wg2T = (io[k] for k in ["xT", "WuT", "WvT", "Wg1T", "Wg2T"])
    outT = io["outT"]

    # ---- constants / small tensors
    tri = const.tile([P, TRI_W], BF)
    nc.sync.dma_start(out=tri, in_=io["tri"][:, :])
    mask = const.tile([NCORES, TC], BF)
    nc.sync.dma_start(out=mask, in_=io["mask"][:, :])
    burow = const.tile([1, R], BF)
    nc.sync.dma_start(out=burow, in_=io["bu_row"][:, :])
    bg2row = const.tile([1, R], BF)
    nc.sync.dma_start(out=bg2row, in_=io["bg2_row"][:, :])
    bvrow = const.tile([1, R], BF)
    nc.sync.dma_start(out=bvrow, in_=io["bv_row"][:, :])
    bg1c = const.tile([P, NDH], F32)
    nc.sync.dma_start(out=bg1c, in_=io["bg1_col"][:].rearrange("(k p) -> p k", p=P))
    bg2c = const.tile([P, NR], F32)
    nc.sync.dma_start(out=bg2c, in_=io["bg2_col"][:].rearrange("(k p) -> p k", p=P))
    cw = const.tile([P, ND, 3], F32)
    nc.sync.dma_start(out=cw, in_=io["conv_w2"][:, :].rearrange("(k p) c -> p k c", p=P))
    cb = const.tile([P, ND], F32)
    nc.sync.dma_start(out=cb, in_=io["conv_b2"][:].rearrange("(k p) -> p k", p=P))
    ones_row = tri[0:1, TRI_OFF : TRI_OFF + P]    # [1, 128] of 1.0
    ones_col = tri[:, ONES_COL : ONES_COL + 1]    # [128, 1] of 1.0
    ones_512 = tri[0:1, TRI_OFF : TRI_OFF + TC]   # [1, 512] of 1.0
    cwtouch = const.tile([P, 1], F32)
    nc.vector.tensor_copy(out=cwtouch, in_=cw[:, 0, 0:1])
    cbtouch = const.tile([P, 1], F32)
    nc.vector.tensor_copy(out=cbtouch, in_=cb[:, 0:1])

    # ---- x shard, transposed, with halo
    xs = []
    for kd in range(ND):
        xpt = xpool.tile([P, TH], BF)
        nc.sync.dma_start(out=xpt, in_=xT[kd * P : (kd + 1) * P, :])
        xs.append(xpt)

    # ---- phase H: hT[dh, t] = relu(Wg1 @ x^T + bg1)
    hs = []
    for kdh in range(NDH):
        hpt = hpool.tile([P, T], BF)
        psh = [psum.tile([P, TC], F32, name=f"psh{c}", tag="ps") for c in range(NTC)]
        for kg in range(ND // 4):
            wg1s = wg1p.tile([P, 4, P], BF)
            nc.sync.dma_start(
                out=wg1s,
                in_=Wg1T[kg * 4 * P : (kg + 1) * 4 * P,
                         kdh * P : (kdh + 1) * P].rearrange("(g p) c -> p g c", p=P),
            )
            for i in range(4):
                k = kg * 4 + i
                for c in range(NTC):
                    nc.tensor.matmul(
                        psh[c],
                        lhsT=wg1s[:, i, :],
                        rhs=xs[k][:, 1 + c * TC : 1 + (c + 1) * TC],
                        start=(k == 0),
                        stop=(k == ND - 1),
                    )
        for c in range(NTC):
            nc.scalar.activation(
                out=hpt[:, c * TC : (c + 1) * TC],
                in_=psh[c],
                func=AF.Relu,
                bias=bg1c[:, kdh : kdh + 1],
            )
        hs.append(hpt)

    # ---- phase G: gates natural [t-part, r-free] = sigmoid(h @ Wg2^T + bg2)
    gn = []
    for t in range(NT):
        gnt = gnp.tile([P, R], BF)
        gn.append(gnt)
    for rc in range(NRC):
        psg = [psum.tile([P, TC], F32, name=f"psg{t}", tag="ps") for t in range(NT)]
        for kdh in range(NDH):
            wg2s = wg2p.tile([P, TC], BF)
            nc.sync.dma_start(
                out=wg2s,
                in_=Wg2T[kdh * P : (kdh + 1) * P, rc * TC : (rc + 1) * TC],
            )
            for t in range(NT):
                nc.tensor.matmul(
                    psg[t],
                    lhsT=hs[kdh][:, t * P : (t + 1) * P],
                    rhs=wg2s,
                    start=(kdh == 0),
                    stop=False,
                )
        for t in range(NT):
            nc.tensor.matmul(
                psg[t],
                lhsT=ones_row,
                rhs=bg2row[:, rc * TC : (rc + 1) * TC],
                start=False,
                stop=True,
            )
            nc.scalar.activation(
                out=gn[t][:, rc * TC : (rc + 1) * TC], in_=psg[t], func=AF.Sigmoid
            )

    if _PHASES < 2:
        for kd in range(ND):
            for c in range(NTC):
                ob = outp.tile([P, TC], F32)
                nc.vector.tensor_copy(out=ob, in_=gn[kd % NT][:, c * TC : (c + 1) * TC])
                nc.sync.dma_start(out=outT[kd * P : (kd + 1) * P, c * TC : (c + 1) * TC], in_=ob)
        ctx.close()
        return

    # ---- phase U: u natural, ug = (u + bu) * gates  (bf16, lhsT of cumsum)
    ugt = [[ugp.tile([P, TC], BF, name=f"ug_{t}_{rc}", tag="ug") for rc in range(NRC)]
           for t in range(NT)]
    for rc in range(NRC):
        psu = [psum.tile([P, TC], F32, name=f"psu{t}", tag="ps") for t in range(NT)]
        for k in range(ND):
            wut = wutp.tile([P, TC], BF)
            nc.sync.dma_start(
                out=wut, in_=WuT[k * P : (k + 1) * P, rc * TC : (rc + 1) * TC]
            )
            for t in range(NT):
                nc.tensor.matmul(
                    psu[t],
                    lhsT=xs[k][:, 1 + t * P : 1 + t * P + P],
                    rhs=wut,
                    start=(k == 0),
                    stop=False,
                )
        for t in range(NT):
            nc.tensor.matmul(
                psu[t],
                lhsT=ones_row,
                rhs=burow[:, rc * TC : (rc + 1) * TC],
                start=False,
                stop=True,
            )
            nc.vector.tensor_mul(
                out=ugt[t][rc], in0=psu[t], in1=gn[t][:, rc * TC : (rc + 1) * TC]
            )

    # ---- block sums bs[r] = sum_t ug[t, r]  (as a [1, R] psum row), then AllGather
    bs_sb = const.tile([P, NR], F32)
    psb = psum.tile([P, TC], F32, name="psb", tag="ps")
    for rk in range(NR):
        rc4, ri4 = rk // 4, rk % 4
        for j in range(NT):
            nc.tensor.matmul(
                psb[:, rk : rk + 1],
                lhsT=ugt[j][rc4][:, ri4 * P : (ri4 + 1) * P],
                rhs=ones_col,
                start=(j == 0),
                stop=(j == NT - 1),
            )
    nc.vector.tensor_copy(out=bs_sb, in_=psb[:, 0:NR])
    agb = const.tile([NCORES, R], BF)
    if os.environ.get("K_NOAG"):
        nc.vector.memset(agb, 0.0)
    else:
        bs_dram = dram.tile([R], F32)
        ag_dram = dram.tile([NCORES * R], F32, addr_space="Shared")
        nc.sync.dma_start(out=bs_dram[:].rearrange("(k p) -> p k", p=P), in_=bs_sb)
        nc.gpsimd.collective_compute(
            "AllGather",
            ALU.bypass,
            replica_groups=[list(range(NCORES))],
            ins=[bs_dram[:].opt()],
            outs=[ag_dram[:].opt()],
        )
        if os.environ.get("K_NOCAST"):
            agf = const.tile([NCORES, R], F32)
            nc.sync.dma_start(out=agf, in_=ag_dram[:].rearrange("(c r) -> c r", c=NCORES))
            nc.vector.tensor_copy(out=agb, in_=agf)
        else:
            nc.gpsimd.dma_start(out=agb, in_=ag_dram[:].rearrange("(c r) -> c r", c=NCORES))

    if _PHASES < 3:
        for kd in range(ND):
            for c in range(NTC):
                ob = outp.tile([P, TC], F32)
                nc.vector.tensor_copy(out=ob, in_=ugt[kd % NT][c % NRC])
                nc.sync.dma_start(out=outT[kd * P : (kd + 1) * P, c * TC : (c + 1) * TC], in_=ob)
        ctx.close()
        return

    # ---- phase V/GT/CS per r-tile: gatesT, vgT, csT (+carry), goT
    gos = []
    for rk in range(NR):
        rc4, ri4 = rk // 4, rk % 4
        # gatesT [r-part, t-free]
        gtt = gtp.tile([P, T], BF)
        wg2t = wg2tp.tile([P, NDH, P], BF)
        nc.sync.dma_start(
            out=wg2t,
            in_=Wg2T[:, rk * P : (rk + 1) * P].rearrange("(g p) c -> p g c", p=P),
        )
        psgt = [psum.tile([P, TC], F32, name=f"psgt{c}", tag="ps") for c in range(NTC)]
        for kdh in range(NDH):
            for c in range(NTC):
                nc.tensor.matmul(
                    psgt[c],
                    lhsT=wg2t[:, kdh, :],
                    rhs=hs[kdh][:, c * TC : (c + 1) * TC],
                    start=(kdh == 0),
                    stop=(kdh == NDH - 1),
                )
        for c in range(NTC):
            nc.scalar.activation(
                out=gtt[:, c * TC : (c + 1) * TC],
                in_=psgt[c],
                func=AF.Sigmoid,
                bias=bg2c[:, rk : rk + 1],
            )
        # vT, vgT = (vT + bv) * gatesT
        vgt = vgp.tile([P, T], BF)
        psv = [psum.tile([P, TC], F32, name=f"psv{c}", tag="ps") for c in range(NTC)]
        for kg in range(ND // 4):
            wvt = wvtp.tile([P, 4, P], BF)
            nc.sync.dma_start(
                out=wvt,
                in_=WvT[kg * 4 * P : (kg + 1) * 4 * P,
                        rk * P : (rk + 1) * P].rearrange("(g p) c -> p g c", p=P),
            )
            for i in range(4):
                k = kg * 4 + i
                for c in range(NTC):
                    nc.tensor.matmul(
                        psv[c],
                        lhsT=wvt[:, i, :],
                        rhs=xs[k][:, 1 + c * TC : 1 + (c + 1) * TC],
                        start=(k == 0),
                        stop=False,
                    )
        for c in range(NTC):
            nc.tensor.matmul(
                psv[c],
                lhsT=bvrow[0:1, rk * P : (rk + 1) * P],
                rhs=ones_512,
                start=False,
                stop=True,
            )
            nc.vector.tensor_mul(
                out=vgt[:, c * TC : (c + 1) * TC],
                in0=psv[c],
                in1=gtt[:, c * TC : (c + 1) * TC],
            )
        # csT via tri matmuls (+ cross-core carry), then goT = csT * vgT
        got = gop.tile([P, T], BF)
        for c in range(NTC):
            pscs = psum.tile([P, TC], F32, name="pscs", tag="ps")
            jmax = min(NT - 1, 4 * c + 3)
            for j in range(jmax + 1):
                off = c * TC - j * P
                nc.tensor.matmul(
                    pscs,
                    lhsT=ugt[j][rc4][:, ri4 * P : (ri4 + 1) * P],
                    rhs=tri[:, off + TRI_OFF : off + TRI_OFF + TC],
                    start=(j == 0),
                    stop=False,
                )
            nc.tensor.matmul(
                pscs,
                lhsT=agb[:, rk * P : (rk + 1) * P],
                rhs=mask,
                start=False,
                stop=True,
            )
            nc.vector.tensor_mul(
                out=got[:, c * TC : (c + 1) * TC],
                in0=pscs,
                in1=vgt[:, c * TC : (c + 1) * TC],
            )
        gos.append(got)

    if _PHASES < 4:
        for kd in range(ND):
            for c in range(NTC):
                ob = outp.tile([P, TC], F32)
                nc.vector.tensor_copy(out=ob, in_=gos[kd][:, c * TC : (c + 1) * TC])
                nc.sync.dma_start(out=outT[kd * P : (kd + 1) * P, c * TC : (c + 1) * TC], in_=ob)
        ctx.close()
        return

    # ---- phase UV + conv epilogue: outT[d, t] = Wu^T-proj + conv + conv_b
    for kd in range(ND):
        wu2 = []
        for rg in range(NR // 4):
            wu24 = wu2p.tile([P, 4, P], BF)
            nc.sync.dma_start(
                out=wu24,
                in_=WuT[rg * 4 * P : (rg + 1) * 4 * P,
                        kd * P : (kd + 1) * P].rearrange("(g p) c -> p g c", p=P),
            )
            wu2.append(wu24)
        for c in range(NTC):
            psuv = psum.tile([P, TC], F32, name="psuv", tag="ps")
            for rk in range(NR):
                nc.tensor.matmul(
                    psuv,
                    lhsT=wu2[rk // 4][:, rk % 4, :],
                    rhs=gos[rk][:, c * TC : (c + 1) * TC],
                    start=(rk == 0),
                    stop=(rk == NR - 1),
                )
            ct = ctp.tile([P, TC], F32)
            nc.vector.tensor_scalar(
                ct,
                xs[kd][:, c * TC : c * TC + TC],
                cw[:, kd, 0:1],
                cb[:, kd : kd + 1],
                op0=ALU.mult,
                op1=ALU.add,
            )
            nc.vector.scalar_tensor_tensor(
                out=ct,
                in0=xs[kd][:, c * TC + 1 : c * TC + 1 + TC],
                scalar=cw[:, kd, 1:2],
                in1=ct,
                op0=ALU.mult,
                op1=ALU.add,
            )
            nc.vector.scalar_tensor_tensor(
                out=ct,
                in0=xs[kd][:, c * TC + 2 : c * TC + 2 + TC],
                scalar=cw[:, kd, 2:3],
                in1=ct,
                op0=ALU.mult,
                op1=ALU.add,
            )
            ob = outp.tile([P, TC], F32)
            nc.vector.tensor_add(out=ob, in0=psuv, in1=ct)
            nc.sync.dma_start(
                out=outT[kd * P : (kd + 1) * P, c * TC : (c + 1) * TC], in_=ob
            )
    ctx.close()


def _split_multi_waits(nc):
    """The walrus build in this env allows only ONE attached sync-wait per
    instruction; hoist extra waits onto standalone InstEventSemaphore ops
    inserted just before, on the same engine (semantically identical)."""
    import bass_rust

    n = 0
    for blk in nc.m.functions[0].blocks:
        changed = False
        out = []
        for ins in blk.instructions:
            si = getattr(ins, "sync_info", None)
            if si is not None and len(si.on_wait) > 1:
                waits = list(si.on_wait)
                for w in waits[:-1]:
                    ev = mybir.InstEventSemaphore(name=f"WSPLIT-{n}", ins=[], outs=[])
                    n += 1
                    ev.engine = ins.engine
                    ev.sync_info = bass_rust.SyncInfo(on_wait=[w], on_update=[])
                    out.append(ev)
                ins.sync_info = bass_rust.SyncInfo(
                    on_wait=[waits[-1]], on_update=list(si.on_update)
                )
                changed = True
            out.append(ins)
        if changed:
            try:
                blk.instructions[:] = out
            except TypeError:
                blk.instructions = out
    return n


def _build():
    nc = bass.Bass(num_devices=NCORES)
    io = {}
    io["xT"] = nc.declare_dram_parameter("xT", [D, TH], BF, False)
    io["WuT"] = nc.declare_dram_parameter("WuT", [D, R], BF, False)
    io["WvT"] = nc.declare_dram_parameter("WvT", [D, R], BF, False)
    io["Wg1T"] = nc.declare_dram_parameter("Wg1T", [D, DH], BF, False)
    io["Wg2T"] = nc.declare_dram_parameter("Wg2T", [DH, R], BF, False)
    io["tri"] = nc.declare_dram_parameter("tri", [P, TRI_W], BF, False)
    io["mask"] = nc.declare_dram_parameter("mask", [NCORES, TC], BF, False)
    io["bu_row"] = nc.declare_dram_parameter("bu_row", [1, R], BF, False)
    io["bg2_row"] = nc.declare_dram_parameter("bg2_row", [1, R], BF, False)
    io["bv_row"] = nc.declare_dram_parameter("bv_row", [1, R], BF, False)
    io["bg1_col"] = nc.declare_dram_parameter("bg1_col", [DH], F32, False)
    io["bg2_col"] = nc.declare_dram_parameter("bg2_col", [R], F32, False)
    io["conv_w2"] = nc.declare_dram_parameter("conv_w2", [D, 3], F32, False)
    io["conv_b2"] = nc.declare_dram_parameter("conv_b2", [D], F32, False)
    io["outT"] = nc.declare_dram_parameter("outT", [D, T], F32, True)
    with tile.TileContext(nc, num_cores=NCORES) as tc:
        io["tc"] = tc
        _emit(nc, io)
    _split_multi_waits(nc)
    return nc


_NC_CACHE = None


def _get_nc():
    global _NC_CACHE
    if _NC_CACHE is None:
        _NC_CACHE = _build()
    return _NC_CACHE


def _prep_in_maps(x, Wu, bu, Wv, bv, Wg1, bg1, Wg2, bg2, conv_w, conv_b):
    bf = ml_dtypes.bfloat16
    f32 = np.float32
    x = np.asarray(x, f32)
    shared = dict(
        WuT=np.asarray(Wu, f32).T.astype(bf),
        WvT=np.asarray(Wv, f32).T.astype(bf),
        Wg1T=np.asarray(Wg1, f32).T.astype(bf),
        Wg2T=np.asarray(Wg2, f32).T.astype(bf),
        tri=(np.arange(P)[:, None] <= (np.arange(TRI_W)[None, :] - TRI_OFF)).astype(bf),
        bu_row=np.asarray(bu, f32).astype(bf).reshape(1, R),
        bg2_row=np.asarray(bg2, f32).astype(bf).reshape(1, R),
        bv_row=np.asarray(bv, f32).astype(bf).reshape(1, R),
        bg1_col=np.ascontiguousarray(np.asarray(bg1, f32)),
        bg2_col=np.ascontiguousarray(np.asarray(bg2, f32)),
        conv_w2=np.ascontiguousarray(np.asarray(conv_w, f32)[:, 0, :]),
        conv_b2=np.ascontiguousarray(np.asarray(conv_b, f32)),
    )
    xflat = x.reshape(B * L, D)
    in_maps = []
    for c in range(NCORES):
        xh = np.zeros((TH, D), f32)
        xh[1 : T + 1] = xflat[c * T : (c + 1) * T]
        if c % 2 == 1:
            xh[0] = xflat[c * T - 1]
        else:
            xh[T + 1] = xflat[(c + 1) * T]
        m = np.zeros((NCORES, TC), f32)
        if c % 2 == 1:
            m[c - 1, :] = 1.0
        in_maps.append(dict(shared, xT=xh.T.astype(bf), mask=m.astype(bf)))
    return in_maps


def _assemble(results):
    out = np.empty((B * L, D), np.float32)
    for c in range(NCORES):
        out[c * T : (c + 1) * T] = np.asarray(results[c]["outT"]).T
    return out.reshape(B, L, D)


def kernel(x, Wu, bu, Wv, bv, Wg1, bg1, Wg2, bg2, conv_w, conv_b):
    in_maps = _prep_in_maps(x, Wu, bu, Wv, bv, Wg1, bg1, Wg2, bg2, conv_w, conv_b)
    res = run_bass_kernel_spmd(_get_nc(), in_maps, core_ids=list(range(NCORES)))
    return _assemble(res.results)


def run_traced(inputs):
    """Profiled run: returns (output, exec_time_ns)."""
    in_maps = _prep_in_maps(**inputs)
    res = run_bass_kernel_spmd(
        _get_nc(), in_maps, core_ids=list(range(NCORES)), trace=True
    )
    return _assemble(res.results), res.exec_time_ns
